# revision 15
# baseline (speedup 1.0000x reference)
"""CubicPchipKANLayer Trainium2 kernel.

Math: out[b,o] = sum_i PCHIP_interp(x[b,i]; knots y[i,:,o]) + bias[o]

Reformulation: with t = clip((x - D_MIN)/H, 0, K-1), the PCHIP interpolant is
linear over the knot tables:
    out[b,o] = sum_{i,k} phi(t[b,i]-k) * 2y[i,o,k] + psi(t[b,i]-k) * Hm[i,o,k]
with phi3(s) = (|s|+0.5)*r^2, psi(s) = s*r^2, r = relu(1-|s|);  m = pchip
slopes (functions of the parameter y only, precomputed host-side).  The
device computes a dense (2*D_IN*K x B) weight matrix on-chip from x and
contracts it with the (2*D_IN*K x D_OUT) tables on the PE.

Sharding: contraction-parallel over d_in — core c owns i in [32c, 32c+32).
Host sums the 8 partial (D_OUT, B) outputs, transposes, adds bias.

Device pipeline, two-pair batches m (8 batches of 4 d_in rows):
  ACT : t = relu(x/H + 31.5); DVE: t = min(t, 63); DMA: t -> DRAM scratch.
  DMA : t rows replicated across partitions (0-stride DRAM source) into an
        s_rep (128, 2B) tile: partition p of pair j holds t[2j + (p>=64)].
  ACT : ab = |s_rep - k|   (k = p mod 64 via per-partition bias)
  DVE : nr = min(ab,1) - 1 = -r
  Pool: r2 = nr*nr         (DVE for the first batches: shorter fill latency)
  DVE : phi3 = (nr+1.5)*r2 ; psi = (s_rep - k)*r2   (two STT ops)
  PE  : 8 accumulating matmuls per batch, tables stationary (128x128),
        W moving (N=512), into two (o_half, B) PSUM accumulators.

HAM note: the NC grants the PE 2.4GHz in k=8 windows and 1.2GHz in k=4
windows (ACT/DVE clocks are fixed); sustained early PE activity opens the
k=8 grant, hence the warmup matmuls.

NOTE: this walrus build allows only ONE semaphore wait per instruction; a
post-scheduling pass splits extra waits onto same-engine NoOps.
"""
import sys
sys.path.insert(0, '/opt/trn_rl_repo')
import numpy as np

B, D_IN, D_OUT, K = 512, 256, 256, 64
D_MIN, D_MAX = -2.0, 2.0
H = (D_MAX - D_MIN) / (K - 1)
N_CORES = 8
I_PER = D_IN // N_CORES          # 32 d_in rows per core
NPAIR = I_PER // 2               # 16 i-pairs per core
NB = NPAIR // 2                  # 8 two-pair batches
BB = 2 * B                       # batched free width

N_WARM = 6                       # PE warmup matmuls (HAM/pstate ramp)
N_R2_DVE = 2                     # first batches compute r2 on DVE (fill latency)

_CACHE = {}


def _pchip_hm(y):
    """H * pchip_slopes(y), float64 internally, mirroring reference._pchip_slopes."""
    y = y.astype(np.float64)
    delta = (y[..., 1:] - y[..., :-1]) / H
    d0, d1 = delta[..., :-1], delta[..., 1:]
    denom = d0 + d1
    small = np.abs(denom) < 1e-12
    hm = 2.0 * d0 * d1 / np.where(small, 1.0, denom)
    hm = np.where(small, 0.0, hm)
    m_inner = np.where(d0 * d1 > 0, hm, 0.0)
    m0 = (3.0 * delta[..., 0] - delta[..., 1]) / 2.0
    mN = (3.0 * delta[..., -1] - delta[..., -2]) / 2.0
    m0 = np.where(m0 * delta[..., 0] <= 0, 0.0, m0)
    mN = np.where(mN * delta[..., -1] <= 0, 0.0, mN)
    cond0 = (delta[..., 0] * delta[..., 1] < 0) & (np.abs(m0) > np.abs(3.0 * delta[..., 0]))
    m0 = np.where(cond0, 3.0 * delta[..., 0], m0)
    condN = (delta[..., -1] * delta[..., -2] < 0) & (np.abs(mN) > np.abs(3.0 * delta[..., -1]))
    mN = np.where(condN, 3.0 * delta[..., -1], mN)
    m = np.concatenate([m0[..., None], m_inner, mN[..., None]], axis=-1)
    return (H * m).astype(np.float32)


def _build_tables(y):
    """Per-core rhs tables, shape (N_CORES, 2*K, 2*NPAIR*D_OUT) fp16.

    Table column group (j, h): h=0 -> +2*y rows for pair j, h=1 -> H*m rows.
    Row layout within a group: 64 k-rows of i0 then 64 k-rows of i1.
    """
    hm = _pchip_hm(y)                                       # (d_in, d_out, K)
    y2 = (2.0 * y.astype(np.float64)).astype(np.float32)
    y2_t = np.ascontiguousarray(np.transpose(y2, (0, 2, 1)))  # (d_in, K, d_out)
    hm_t = np.ascontiguousarray(np.transpose(hm, (0, 2, 1)))
    tbl = np.empty((N_CORES, NPAIR, 2, 2, K, D_OUT), np.float32)
    for c in range(N_CORES):
        i0 = c * I_PER
        tbl[c, :, 0] = y2_t[i0:i0 + I_PER].reshape(NPAIR, 2, K, D_OUT)
        tbl[c, :, 1] = hm_t[i0:i0 + I_PER].reshape(NPAIR, 2, K, D_OUT)
    # (c, j, h, half, k, o) -> rows (half,k) x cols (j,h,o)
    tbl = tbl.transpose(0, 3, 4, 1, 2, 5).reshape(N_CORES, 2 * K, 2 * NPAIR * D_OUT)
    return np.ascontiguousarray(tbl.astype(np.float16))


def _build_bass():
    import concourse.bass as bass
    import concourse.tile as tile
    from concourse import mybir

    F32 = mybir.dt.float32
    F16 = mybir.dt.float16
    ACTF = mybir.ActivationFunctionType
    ALU = mybir.AluOpType
    TW = 2 * NPAIR * D_OUT            # 8192 table columns

    nc = bass.Bass()
    xt_d = nc.dram_tensor("xt", [32, B], F32, kind="ExternalInput")
    tbl_d = nc.dram_tensor("tbl", [2 * K, TW], F16, kind="ExternalInput")
    k_d = nc.dram_tensor("kcol", [128, 2], F32, kind="ExternalInput")
    # t staged in DRAM regrouped as t2[h, r*B+b] = t[2r+h, b] so one
    # 0-stride-broadcast DMA replicates a whole two-pair batch
    t_d = nc.dram_tensor("tscratch", [2, NB * BB], F32, kind="Internal")
    out_d = nc.dram_tensor("out", [D_OUT, B], F32, kind="ExternalOutput")

    with tile.TileContext(nc) as tc:
        with tc.tile_pool(name="const", bufs=1) as cpool, \
             tc.tile_pool(name="sr", bufs=3) as srpool, \
             tc.tile_pool(name="wk", bufs=3) as wkpool, \
             tc.tile_pool(name="wt", bufs=3) as wtpool, \
             tc.tile_pool(name="pacc", bufs=1, space="PSUM") as paccpool, \
             tc.tile_pool(name="pbc", bufs=1, space="PSUM") as pbcpool:

            # xt + kcol first on the HWDGE ring so t-prep can start ASAP
            xt_t = cpool.tile([32, B], F32)
            nc.sync.dma_start(xt_t[:], xt_d[:])
            k_t = cpool.tile([128, 2], F32)
            nc.sync.dma_start(k_t[:], k_d[:])
            tbl_t = cpool.tile([2 * K, TW], F16)
            for p in range(8):
                w = TW // 8
                nc.sync.dma_start(tbl_t[:, p * w:(p + 1) * w],
                                  tbl_d[:, p * w:(p + 1) * w])

            # constants via memset (no DMA)
            tc_t = cpool.tile([32, 1], F32)
            nc.gpsimd.memset(tc_t[:], -D_MIN / H)
            warm_t = cpool.tile([128, B], F16)
            nc.gpsimd.memset(warm_t[:], 0.0)

            # PE warmup: sustained early activity opens the HAM k=8 grant
            wacc = pbcpool.tile([128, B], F32, tag="wa", name="wacc")
            for _ in range(N_WARM):
                nc.tensor.matmul(wacc[:], warm_t[:, 0:128], warm_t[:],
                                 start=True, stop=True)

            # t = clip(x/H - D_MIN/H, 0, K-1), then to DRAM for replication
            t_t = cpool.tile([32, B], F32)
            nc.scalar.activation(t_t[:], xt_t[:], ACTF.Relu,
                                 bias=tc_t[:], scale=1.0 / H)
            nc.vector.tensor_scalar_min(t_t[:], t_t[:], float(K - 1))
            for h in (0, 1):
                nc.sync.dma_start(t_d[h:h + 1, :], t_t[h:32:2, :])

            accT = [paccpool.tile([128, B], F32, tag=f"accT{q}", name=f"accT{q}")
                    for q in range(2)]

            def replicate(m):
                # s_rep (128, 2B): partition p = hlf*64 + k holds t-row
                # 4m+2u+hlf at col group u; ONE 0-stride DRAM-source DMA
                s_rep = srpool.tile([128, BB], F32, tag="sr")
                src = t_d[:, m * BB:(m + 1) * BB].unsqueeze(1) \
                         .broadcast_to([2, 64, BB])
                nc.sync.dma_start(s_rep[:], src)
                return s_rep

            def elemwise(m, s_rep):
                # ab = |s - k| on ACT (bias = -k per partition)
                ab_t = wkpool.tile([128, BB], F16, tag="ab")
                nc.scalar.activation(ab_t[:], s_rep[:], ACTF.Abs,
                                     bias=k_t[:, 0:1])
                # nr = min(ab,1) - 1 = -r  (DVE, all-SBUF fp16, 4x mode)
                nr_t = wkpool.tile([128, BB], F16, tag="nr")
                nc.vector.tensor_scalar(nr_t[:], ab_t[:], 1.0, -1.0,
                                        op0=ALU.min, op1=ALU.add)
                # r2 = nr^2: Pool steady-state, DVE for the first batches
                r2_t = wkpool.tile([128, BB], F16, tag="r2")
                if m < N_R2_DVE:
                    nc.vector.tensor_mul(r2_t[:], nr_t[:], nr_t[:])
                else:
                    nc.gpsimd.tensor_mul(r2_t[:], nr_t[:], nr_t[:])
                # w tile: phi3 = (nr+1.5)*r2 cols [0,BB), psi = (s-k)*r2 rest
                w_t = wtpool.tile([128, 2 * BB], F16, tag="w")
                nc.vector.scalar_tensor_tensor(w_t[:, 0:BB], nr_t[:], 1.5,
                                               r2_t[:], op0=ALU.add, op1=ALU.mult)
                nc.vector.scalar_tensor_tensor(w_t[:, BB:2 * BB], s_rep[:],
                                               k_t[:, 1:2], r2_t[:],
                                               op0=ALU.subtract, op1=ALU.mult)
                return w_t

            def mains(m, w_t):
                # pair j=2m+u: phi at w[:, u*B:(u+1)*B], psi at w[:, BB+u*B:...]
                for u in (0, 1):
                    j = 2 * m + u
                    for h in (0, 1):
                        base = (j * 2 + h) * D_OUT
                        src_w = w_t[:, h * BB + u * B: h * BB + (u + 1) * B]
                        for q in range(2):
                            nc.tensor.matmul(
                                accT[q][:],
                                tbl_t[:, base + q * 128: base + (q + 1) * 128],
                                src_w,
                                start=(j == 0 and h == 0),
                                stop=(j == NPAIR - 1 and h == 1))

            prev = None
            for m in range(NB):
                s_rep = replicate(m)
                w_t = elemwise(m, s_rep)
                if prev is not None:
                    mains(m - 1, prev)
                prev = w_t
            mains(NB - 1, prev)

            # stage PSUM->SBUF on two different engines, then DMA out
            o0 = cpool.tile([128, B], F32, name="o0")
            o1 = cpool.tile([128, B], F32, name="o1")
            nc.scalar.copy(o0[:], accT[0][:])
            nc.vector.tensor_scalar_add(o1[:], accT[1][:], 0.0)
            nc.sync.dma_start(out_d[0:128, :], o0[:])
            nc.sync.dma_start(out_d[128:256, :], o1[:])

    return nc


def _split_multiwaits(nc):
    """Walrus in this build allows one semaphore wait per instruction.  Tile
    sometimes emits several; split the extras onto same-engine NoOps inserted
    immediately before the instruction (queue order preserves semantics)."""
    from concourse import mybir

    fix_id = 0
    for f in nc.m.functions:
        for blk in f.blocks:
            insts = blk.instructions
            out, changed = [], False
            for ins in insts:
                si = getattr(ins, "sync_info", None)
                waits = list(si.on_wait) if si and si.on_wait else []
                if len(waits) > 1:
                    for w in waits[:-1]:
                        nop = mybir.InstNoOp(name=f"I-fixw{fix_id}",
                                             engine=ins.engine)
                        fix_id += 1
                        nop.sync_info = mybir.SyncInfo(on_wait=[w], on_update=[])
                        out.append(nop)
                    ins.sync_info = mybir.SyncInfo(
                        on_wait=[waits[-1]], on_update=list(si.on_update))
                    changed = True
                out.append(ins)
            if changed:
                blk.instructions = out
    return nc


def _get_compiled():
    if "nc" not in _CACHE:
        nc = _build_bass()
        _split_multiwaits(nc)
        _CACHE["nc"] = nc
    return _CACHE["nc"]


def _run(x, y, bias, trace=False):
    from concourse.bass_utils import run_bass_kernel_spmd

    x = np.asarray(x, np.float32)
    y = np.asarray(y, np.float32)
    bias = np.asarray(bias, np.float32)

    nc = _get_compiled()

    xs = np.ascontiguousarray(x.T)                     # (d_in, B)
    tbl = _build_tables(y)                             # (8, 128, 8192)
    kcol = np.empty((128, 2), np.float32)
    kcol[:, 0] = -(np.arange(128) % 64)                # ACT bias: -k
    kcol[:, 1] = (np.arange(128) % 64)                 # psi scalar: +k

    in_maps = []
    for c in range(N_CORES):
        in_maps.append({
            "xt": np.ascontiguousarray(xs[c * I_PER:(c + 1) * I_PER]),
            "tbl": tbl[c],
            "kcol": kcol,
        })
    res = run_bass_kernel_spmd(nc, in_maps, core_ids=list(range(N_CORES)),
                               trace=trace)
    partialT = np.stack([res.results[c]["out"] for c in range(N_CORES)])
    out = partialT.astype(np.float64).sum(axis=0).T + bias.astype(np.float64)
    return out.astype(np.float32), res


def kernel(x, y, bias):
    out, _ = _run(x, y, bias)
    return out


# revision 16
# speedup vs baseline: 2.1854x; 2.1854x over previous
"""CubicPchipKANLayer Trainium2 kernel.

Math: out[b,o] = sum_i PCHIP_interp(x[b,i]; knots y[i,:,o]) + bias[o]

Reformulation: with t = clip((x - D_MIN)/H, 0, K-1), the PCHIP interpolant is
linear over the knot tables:
    out[b,o] = sum_{i,k} phi(t[b,i]-k) * 2y[i,o,k] + psi(t[b,i]-k) * Hm[i,o,k]
with phi3(s) = (|s|+0.5)*r^2, psi(s) = s*r^2, r = relu(1-|s|);  m = pchip
slopes (functions of the parameter y only, precomputed host-side).  The
device computes a dense (2*D_IN*K x B) weight matrix on-chip from x and
contracts it with the (2*D_IN*K x D_OUT) tables on the PE.

Sharding: contraction-parallel over d_in — core c owns i in [32c, 32c+32).
Host sums the 8 partial (D_OUT, B) outputs, transposes, adds bias.

Device pipeline, two-pair batches m (8 batches of 4 d_in rows):
  PE  : s = E_j^T @ [t_hi; t_lo; 1] for the two pairs of the batch into one
        (128, 2B) two-bank PSUM tile (E carries a -k row; c=65 fp16 matmul
        reconstructs t to ~2^-22).
  ACT : ab = |s|;  DVE: nr = min(ab,1) - 1 = -r
  Pool: r2 = nr*nr  (DVE on the first batches: shorter pipeline fill)
  DVE : phi3 = (nr+1.5)*r2 (STT); psi = s*r2 (TT from PSUM)
  PE  : 8 accumulating matmuls per batch, tables stationary (128x128),
        W moving (N=512), into two (o_half, B) PSUM accumulators.

HAM note: the NC grants the PE 2.4GHz in k=8 windows and 1.2GHz in k=4
windows (ACT/DVE clocks are fixed); sustained early PE activity opens the
k=8 grant, hence the warmup matmuls.

NOTE: this walrus build allows only ONE semaphore wait per instruction; a
post-scheduling pass splits extra waits onto same-engine NoOps.
"""
import sys
sys.path.insert(0, '/opt/trn_rl_repo')
import numpy as np

B, D_IN, D_OUT, K = 512, 256, 256, 64
D_MIN, D_MAX = -2.0, 2.0
H = (D_MAX - D_MIN) / (K - 1)
N_CORES = 8
I_PER = D_IN // N_CORES          # 32 d_in rows per core
NPAIR = I_PER // 2               # 16 i-pairs per core
NB = NPAIR // 2                  # 8 two-pair batches
BB = 2 * B                       # batched free width

N_WARM = 10                      # PE warmup matmuls (HAM/pstate ramp)
N_R2_DVE = 2                     # first batches compute r2 on DVE (fill latency)

_CACHE = {}


def _pchip_hm(y):
    """H * pchip_slopes(y), float64 internally, mirroring reference._pchip_slopes."""
    y = y.astype(np.float64)
    delta = (y[..., 1:] - y[..., :-1]) / H
    d0, d1 = delta[..., :-1], delta[..., 1:]
    denom = d0 + d1
    small = np.abs(denom) < 1e-12
    hm = 2.0 * d0 * d1 / np.where(small, 1.0, denom)
    hm = np.where(small, 0.0, hm)
    m_inner = np.where(d0 * d1 > 0, hm, 0.0)
    m0 = (3.0 * delta[..., 0] - delta[..., 1]) / 2.0
    mN = (3.0 * delta[..., -1] - delta[..., -2]) / 2.0
    m0 = np.where(m0 * delta[..., 0] <= 0, 0.0, m0)
    mN = np.where(mN * delta[..., -1] <= 0, 0.0, mN)
    cond0 = (delta[..., 0] * delta[..., 1] < 0) & (np.abs(m0) > np.abs(3.0 * delta[..., 0]))
    m0 = np.where(cond0, 3.0 * delta[..., 0], m0)
    condN = (delta[..., -1] * delta[..., -2] < 0) & (np.abs(mN) > np.abs(3.0 * delta[..., -1]))
    mN = np.where(condN, 3.0 * delta[..., -1], mN)
    m = np.concatenate([m0[..., None], m_inner, mN[..., None]], axis=-1)
    return (H * m).astype(np.float32)


def _build_tables(y):
    """Per-core rhs tables, shape (N_CORES, 2*K, 2*NPAIR*D_OUT) fp16.

    Table column group (j, h): h=0 -> +2*y rows for pair j, h=1 -> H*m rows.
    Row layout within a group: 64 k-rows of i0 then 64 k-rows of i1.
    """
    hm = _pchip_hm(y)                                       # (d_in, d_out, K)
    y2 = (2.0 * y.astype(np.float64)).astype(np.float32)
    y2_t = np.ascontiguousarray(np.transpose(y2, (0, 2, 1)))  # (d_in, K, d_out)
    hm_t = np.ascontiguousarray(np.transpose(hm, (0, 2, 1)))
    tbl = np.empty((N_CORES, NPAIR, 2, 2, K, D_OUT), np.float32)
    for c in range(N_CORES):
        i0 = c * I_PER
        tbl[c, :, 0] = y2_t[i0:i0 + I_PER].reshape(NPAIR, 2, K, D_OUT)
        tbl[c, :, 1] = hm_t[i0:i0 + I_PER].reshape(NPAIR, 2, K, D_OUT)
    # (c, j, h, half, k, o) -> rows (half,k) x cols (j,h,o)
    tbl = tbl.transpose(0, 3, 4, 1, 2, 5).reshape(N_CORES, 2 * K, 2 * NPAIR * D_OUT)
    return np.ascontiguousarray(tbl.astype(np.float16))



def _build_selector():
    """E (65, NPAIR*128) fp16: per pair j a (65,128) stationary block.
    Rows 0-31 select t_hi rows (1.0 where (p<64, c==2j) or (p>=64, c==2j+1)),
    rows 32-63 repeat the selector for the t_lo rows, row 64 is -(p mod 64)
    (pairs with the ones-row).  All entries are fp16-exact (ints <= 63)."""
    e = np.zeros((65, NPAIR * 128), np.float16)
    for j in range(NPAIR):
        e[2 * j, j * 128:j * 128 + 64] = 1.0
        e[2 * j + 1, j * 128 + 64:(j + 1) * 128] = 1.0
        e[32 + 2 * j, j * 128:j * 128 + 64] = 1.0
        e[32 + 2 * j + 1, j * 128 + 64:(j + 1) * 128] = 1.0
    e[64] = np.tile(-(np.arange(128, dtype=np.float16) % 64), NPAIR)
    return e

def _build_bass():
    import concourse.bass as bass
    import concourse.tile as tile
    from concourse import mybir

    F32 = mybir.dt.float32
    F16 = mybir.dt.float16
    ACTF = mybir.ActivationFunctionType
    ALU = mybir.AluOpType
    TW = 2 * NPAIR * D_OUT            # 8192 table columns

    nc = bass.Bass()
    xt_d = nc.dram_tensor("xt", [33, B], F32, kind="ExternalInput")
    tbl_d = nc.dram_tensor("tbl", [2 * K, TW], F16, kind="ExternalInput")
    e_d = nc.dram_tensor("sel", [65, NPAIR * 128], F16, kind="ExternalInput")
    out_d = nc.dram_tensor("out", [D_OUT, B], F32, kind="ExternalOutput")

    with tile.TileContext(nc) as tc:
        with tc.tile_pool(name="const", bufs=1) as cpool, \
             tc.tile_pool(name="wk", bufs=3) as wkpool, \
             tc.tile_pool(name="wt", bufs=2) as wtpool, \
             tc.tile_pool(name="pacc", bufs=1, space="PSUM") as paccpool, \
             tc.tile_pool(name="pbc", bufs=3, space="PSUM") as pbcpool:

            # xt first on the HWDGE ring so t-prep can start ASAP
            xt_t = cpool.tile([33, B], F32)
            nc.sync.dma_start(xt_t[:], xt_d[:])
            e_t = cpool.tile([65, NPAIR * 128], F16)
            nc.sync.dma_start(e_t[:], e_d[:])
            tbl_t = cpool.tile([2 * K, TW], F16)
            for p in range(8):
                w = TW // 8
                nc.sync.dma_start(tbl_t[:, p * w:(p + 1) * w],
                                  tbl_d[:, p * w:(p + 1) * w])

            # constants via memset (no DMA)
            tc_t = cpool.tile([33, 1], F32)
            nc.gpsimd.memset(tc_t[0:32, :], -D_MIN / H)
            nc.gpsimd.memset(tc_t[32:33, :], 0.0)
            warm_t = cpool.tile([128, B], F16)
            nc.gpsimd.memset(warm_t[:], 0.0)

            # PE warmup: sustained early activity opens the HAM k=8 grant
            wacc = pbcpool.tile([128, BB], F32, tag="bc", name="wacc")
            for _ in range(N_WARM):
                nc.tensor.matmul(wacc[:, 0:B], warm_t[:, 0:128], warm_t[:],
                                 start=True, stop=True)

            # t = clip(x/H - D_MIN/H, 0, K-1); row 32 becomes exactly 1.0
            t_t = cpool.tile([33, B], F32)
            nc.scalar.activation(t_t[:], xt_t[:], ACTF.Relu,
                                 bias=tc_t[:], scale=1.0 / H)
            nc.vector.tensor_scalar_min(t_t[0:32, :], t_t[0:32, :], float(K - 1))
            # t2: rows 0-31 fp16 hi, 32-63 lo, 64 ones (constant, via memset)
            t2_t = cpool.tile([65, B], F16)
            nc.gpsimd.memset(t2_t[64:65, :], 1.0)
            nc.vector.tensor_scalar_add(t2_t[0:32, :], t_t[0:32, :], 0.0)
            nc.vector.tensor_sub(t2_t[32:64, :], t_t[0:32, :], t2_t[0:32, :])

            accT = [paccpool.tile([128, B], F32, tag=f"accT{q}", name=f"accT{q}")
                    for q in range(2)]

            def bcast2(m):
                # s for pairs (2m, 2m+1) into one (128, 2B) two-bank tile
                bacc = pbcpool.tile([128, BB], F32, tag="bc")
                for u in (0, 1):
                    j = 2 * m + u
                    nc.tensor.matmul(bacc[:, u * B:(u + 1) * B],
                                     e_t[:, j * 128:(j + 1) * 128],
                                     t2_t[:], start=True, stop=True)
                return bacc

            def elemwise(m, bacc):
                # ab = |s| on ACT (the PSUM reader; even piece)
                ab_t = wkpool.tile([128, BB], F16, tag="ab")
                nc.scalar.activation(ab_t[:], bacc[:], ACTF.Abs)
                # nr = min(ab,1) - 1 = -r  (DVE, all-SBUF fp16, 4x mode)
                nr_t = wkpool.tile([128, BB], F16, tag="nr")
                nc.vector.tensor_scalar(nr_t[:], ab_t[:], 1.0, -1.0,
                                        op0=ALU.min, op1=ALU.add)
                # r2 = nr^2: Pool steady-state, DVE for the first batches
                r2_t = wkpool.tile([128, BB], F16, tag="r2")
                if m < N_R2_DVE:
                    nc.vector.tensor_mul(r2_t[:], nr_t[:], nr_t[:])
                else:
                    nc.gpsimd.tensor_mul(r2_t[:], nr_t[:], nr_t[:])
                # w tile: phi3 = (nr+1.5)*r2 cols [0,BB), psi = (s-k)*r2 rest
                w_t = wtpool.tile([128, 2 * BB], F16, tag="w")
                nc.vector.scalar_tensor_tensor(w_t[:, 0:BB], nr_t[:], 1.5,
                                               r2_t[:], op0=ALU.add, op1=ALU.mult)
                nc.vector.tensor_mul(w_t[:, BB:2 * BB], bacc[:], r2_t[:])
                return w_t

            def mains(m, w_t):
                # pair j=2m+u: phi at w[:, u*B:(u+1)*B], psi at w[:, BB+u*B:...]
                for u in (0, 1):
                    j = 2 * m + u
                    for h in (0, 1):
                        base = (j * 2 + h) * D_OUT
                        src_w = w_t[:, h * BB + u * B: h * BB + (u + 1) * B]
                        for q in range(2):
                            nc.tensor.matmul(
                                accT[q][:],
                                tbl_t[:, base + q * 128: base + (q + 1) * 128],
                                src_w,
                                start=(j == 0 and h == 0),
                                stop=(j == NPAIR - 1 and h == 1))

            prev = None
            for m in range(NB):
                bacc = bcast2(m)
                w_t = elemwise(m, bacc)
                if prev is not None:
                    mains(m - 1, prev)
                prev = w_t
            mains(NB - 1, prev)

            # stage PSUM->SBUF on two different engines, then DMA out
            o0 = cpool.tile([128, B], F32, name="o0")
            o1 = cpool.tile([128, B], F32, name="o1")
            nc.scalar.copy(o0[:], accT[0][:])
            nc.vector.tensor_scalar_add(o1[:], accT[1][:], 0.0)
            nc.sync.dma_start(out_d[0:128, :], o0[:])
            nc.sync.dma_start(out_d[128:256, :], o1[:])

    return nc


def _split_multiwaits(nc):
    """Walrus in this build allows one semaphore wait per instruction.  Tile
    sometimes emits several; split the extras onto same-engine NoOps inserted
    immediately before the instruction (queue order preserves semantics)."""
    from concourse import mybir

    fix_id = 0
    for f in nc.m.functions:
        for blk in f.blocks:
            insts = blk.instructions
            out, changed = [], False
            for ins in insts:
                si = getattr(ins, "sync_info", None)
                waits = list(si.on_wait) if si and si.on_wait else []
                if len(waits) > 1:
                    for w in waits[:-1]:
                        nop = mybir.InstNoOp(name=f"I-fixw{fix_id}",
                                             engine=ins.engine)
                        fix_id += 1
                        nop.sync_info = mybir.SyncInfo(on_wait=[w], on_update=[])
                        out.append(nop)
                    ins.sync_info = mybir.SyncInfo(
                        on_wait=[waits[-1]], on_update=list(si.on_update))
                    changed = True
                out.append(ins)
            if changed:
                blk.instructions = out
    return nc


def _get_compiled():
    if "nc" not in _CACHE:
        nc = _build_bass()
        _split_multiwaits(nc)
        _CACHE["nc"] = nc
    return _CACHE["nc"]


def _run(x, y, bias, trace=False):
    from concourse.bass_utils import run_bass_kernel_spmd

    x = np.asarray(x, np.float32)
    y = np.asarray(y, np.float32)
    bias = np.asarray(bias, np.float32)

    nc = _get_compiled()

    xs = np.ascontiguousarray(x.T)                     # (d_in, B)
    tbl = _build_tables(y)                             # (8, 128, 8192)
    e_np = _build_selector()

    in_maps = []
    for c in range(N_CORES):
        xt = np.empty((33, B), np.float32)
        xt[:32] = xs[c * I_PER:(c + 1) * I_PER]
        xt[32] = H                                     # relu(H/H + 0) == 1.0
        in_maps.append({
            "xt": xt,
            "tbl": tbl[c],
            "sel": e_np,
        })
    res = run_bass_kernel_spmd(nc, in_maps, core_ids=list(range(N_CORES)),
                               trace=trace)
    partialT = np.stack([res.results[c]["out"] for c in range(N_CORES)])
    out = partialT.astype(np.float64).sum(axis=0).T + bias.astype(np.float64)
    return out.astype(np.float32), res


def kernel(x, y, bias):
    out, _ = _run(x, y, bias)
    return out
